# revision 17
# baseline (speedup 1.0000x reference)
"""Trainium2 Bass kernel for the sparse_attention nn_Kernel problem.

Math (per sample, derived from the reference):
  t1 = p1w * x ; t2 = roll(t1, 1, ch) ; t3_k = shift_{k-3}(t2) (zero-padded, w)
  C_k[i,m] = sum_p x[i,p] * t1pad[m, p+s]   (s = k-3; c x c)
  t7m_k[q=m, i] = A[q] - C_k^T[q]  where A[q] = C_3^T[q+1] (partition roll)
  Sm[q=m] = 7*A[q] - sum_k C_k^T[q] ; Sj[q] = Sm[q-1]
  out = roll_{h+1,w-1}( sum_k t7m_k^T @ t1pad(col shift s) + Sj^T @ invroll_hw(x) )

Key layout/schedule tricks vs the original baseline:
  - the 7 shifted transposed operands t3_kT[p',m] = t1pad[m, p'+s] are built by
    XBAR DMA transposes reading the t1 buffer at a free-dim column offset s
    (slack zeros give exact unfold semantics), NOT via a DRAM partition-shift
    bounce.  This halves DMA traffic and removes ~120us of serialized queue
    time.  All XBAR transposes stay on ONE queue (sync): concurrent XBAR
    transposes on both hwdge queues corrupt each other (shared XBAR unit).
  - plain DRAM loads/stores and small partition-roll copies go on the OTHER
    hwdge queue (scalar) and overlap the XBAR phase safely.
  - bmm2's ib=0 accumulation is interleaved into the bmm1 k-loop (7 PSUM
    Q-tiles stay open across k), so the XBAR-transpose-bound phase also
    retires bmm2 work; ib=1 runs as a second pass from the persistent t7
    tiles.
  - pad-only memsets, issued before the loads; element-wise work split
    across vector / scalar / gpsimd; final cyclic roll folded into the
    PSUM->SBUF copies.

Each of the 8 cores processes one sample of the batch (data parallel).
"""

import math

import numpy as np

C = 256
H = 56
W = 56
WP = 64  # padded width
PADL = 3
NPP = H * WP  # 3584 padded positions
NCH = NPP // 128  # 28 chunks of 128 partitions
K = 7
SL = 8  # slack zero cols each side of t1buf so shifted transposes stay in range
BETA = 1.0 / (math.sqrt(H * W) * math.sqrt(C * K))
N_CORES = 8
HT = 8  # h rows per bmm2 out tile
NQT = H // HT  # 7 tiles per ib

_CACHE = {}


def _build_nc():
    import concourse.mybir as mybir
    import concourse.tile as tile
    from concourse import bacc

    f32 = mybir.dt.float32
    bf16 = mybir.dt.bfloat16

    nc = bacc.Bacc("TRN2", target_bir_lowering=False, debug=False)

    xin = nc.dram_tensor("x", [C, H, W], f32, kind="ExternalInput").ap()
    pwin = nc.dram_tensor("p1w", [C, H, W], f32, kind="ExternalInput").ap()
    out = nc.dram_tensor("out", [C, H, W], f32, kind="ExternalOutput").ap()

    sub = mybir.AluOpType.subtract
    mult = mybir.AluOpType.mult
    add = mybir.AluOpType.add

    with tile.TileContext(nc) as tc:
        with (
            tc.tile_pool(name="f32big", bufs=1) as pf32,
            tc.tile_pool(name="bfbig", bufs=1) as pbf,
            tc.tile_pool(name="bfroll", bufs=1) as pbr,
            tc.tile_pool(name="ptrans", bufs=1) as pxT,
            tc.tile_pool(name="pt3", bufs=3) as pt3,
            tc.tile_pool(name="small", bufs=1) as psm,
            tc.tile_pool(name="ps1", bufs=2, space="PSUM") as pps1,
            tc.tile_pool(name="ps2", bufs=6, space="PSUM") as pps2,
        ):
            # ------------- tiles + pad memsets (before loads) -------------
            x_cp, p_cp, x_bf, t1buf = [], [], [], []
            for cb in range(2):
                xt = pf32.tile([128, H, W], f32, tag=f"xcp{cb}")
                x_cp.append(xt)
                pt = pf32.tile([128, H, W], f32, tag=f"pcp{cb}")
                p_cp.append(pt)

                tb = pbf.tile([128, 2 * SL + NPP], bf16, tag=f"t1b{cb}")
                nc.vector.memset(tb[:, 0:SL], 0.0)
                nc.vector.memset(tb[:, SL + NPP : 2 * SL + NPP], 0.0)
                tb3 = tb[:, SL : SL + NPP].rearrange("p (h w) -> p h w", w=WP)
                nc.vector.memset(tb3[:, :, 0:PADL], 0.0)
                nc.vector.memset(tb3[:, :, PADL + W : WP], 0.0)
                t1buf.append(tb)

                xb = pbf.tile([128, NPP], bf16, tag=f"xbf{cb}")
                xb3 = xb.rearrange("p (h w) -> p h w", w=WP)
                nc.vector.memset(xb3[:, :, 0:PADL], 0.0)
                nc.vector.memset(xb3[:, :, PADL + W : WP], 0.0)
                x_bf.append(xb)

            # ------------- loads (scalar hwdge queue; sync = XBAR only) ----
            for cb in range(2):
                nc.sync.dma_start(x_cp[cb][:], xin[cb * 128 : (cb + 1) * 128])
                nc.sync.dma_start(p_cp[cb][:], pwin[cb * 128 : (cb + 1) * 128])

            # ------------- padded bf16 operands ----------------------------
            for cb in range(2):
                tb3 = t1buf[cb][:, SL : SL + NPP].rearrange(
                    "p (h w) -> p h w", w=WP
                )
                nc.vector.tensor_mul(
                    tb3[:, :, PADL : PADL + W], x_cp[cb][:], p_cp[cb][:]
                )
                xb3 = x_bf[cb].rearrange("p (h w) -> p h w", w=WP)
                nc.vector.tensor_scalar_mul(
                    xb3[:, :, PADL : PADL + W], x_cp[cb][:], BETA
                )

            def t1view(cb):
                return t1buf[cb][:, SL : SL + NPP].rearrange(
                    "p (h w) -> p h w", w=WP
                )

            # ------------- transposes (XBAR, sync queue ONLY) --------------
            t1T = pxT.tile([128, NCH, C], bf16, tag="t1T")
            xpT = pxT.tile([128, NCH, C], bf16, tag="xpT")
            for cb in range(2):
                nc.sync.dma_start_transpose(
                    t1T[:, :, cb * 128 : (cb + 1) * 128],
                    t1buf[cb][:, SL : SL + NPP],
                )
                nc.sync.dma_start_transpose(
                    xpT[:, :, cb * 128 : (cb + 1) * 128], x_bf[cb][:, :]
                )

            def make_t3T(k):
                s = k - 3
                t3k = pt3.tile([128, NCH, C], bf16, tag="t3")
                for cb in range(2):
                    nc.sync.dma_start_transpose(
                        t3k[:, :, cb * 128 : (cb + 1) * 128],
                        t1buf[cb][:, SL + s : SL + s + NPP],
                    )
                return t3k

            # xroll[j, h', w'] = x[j, (h'+1)%H, (w'-1)%W]  (bf16, for S-term;
            # vector, after the transpose-critical prep)
            xroll = []
            for cb in range(2):
                xr = pbr.tile([128, H, W], bf16, tag=f"xroll{cb}")
                nc.vector.tensor_copy(
                    out=xr[:, 0:55, 1:W], in_=x_cp[cb][:, 1:56, 0 : W - 1]
                )
                nc.vector.tensor_copy(
                    out=xr[:, 0:55, 0:1], in_=x_cp[cb][:, 1:56, W - 1 : W]
                )
                nc.vector.tensor_copy(
                    out=xr[:, 55:56, 1:W], in_=x_cp[cb][:, 0:1, 0 : W - 1]
                )
                nc.vector.tensor_copy(
                    out=xr[:, 55:56, 0:1], in_=x_cp[cb][:, 0:1, W - 1 : W]
                )
                xroll.append(xr)

            # ------------- bmm1 set helper ---------------------------------
            def bmm1_set(Tw):
                tiles = []
                for mb in range(2):
                    pt = pps1.tile([128, C], f32, tag="ps1")
                    for t in range(NCH):
                        nc.tensor.matmul(
                            pt[:],
                            Tw[:, t, mb * 128 : mb * 128 + 128],
                            xpT[:, t, :],
                            start=(t == 0),
                            stop=(t == NCH - 1),
                        )
                    tiles.append(pt)
                return tiles

            # ------------- bmm2 ib=0 Q tiles (persist across the k loop) ---
            NQ0 = 6  # tiles fused into the k loop (PSUM bank budget)
            q_tiles0 = []
            for j in range(NQ0):
                q = pps2.tile([128, HT * W], f32, tag="ps2")
                q_tiles0.append(q.rearrange("p (h w) -> p h w", w=W))

            h0s0 = [j * HT for j in range(NQ0)]
            t7 = {}

            def bmm2_partial(k, qviews, h0s, ib, first, last=False):
                s = k - 3
                for mb in range(2):
                    for qi, (qv, h0) in enumerate(zip(qviews, h0s)):
                        rhs = t1view(mb)[
                            :, h0 : h0 + HT, PADL + s : PADL + s + W
                        ]
                        nc.tensor.matmul(
                            qv[:],
                            t7[(k, mb)][:, ib * 128 : (ib + 1) * 128],
                            rhs,
                            start=(first and mb == 0),
                            stop=(last and mb == 1),
                        )

            def bmm2_sterm(qviews, h0s, ib):
                for mb in range(2):
                    for qv, h0 in zip(qviews, h0s):
                        nc.tensor.matmul(
                            qv[:],
                            Sj_bf[mb][:, ib * 128 : (ib + 1) * 128],
                            xroll[mb][:, h0 : h0 + HT, :],
                            start=(mb == 0),
                            stop=(mb == 1),
                        )

            # ------------- C_3, A roll, then k loop with fused bmm2 --------
            C3_ps = bmm1_set(t1T)
            C3_sb = []
            for mb in range(2):
                c3 = psm.tile([128, C], f32, tag=f"c3sb{mb}")
                nc.vector.tensor_copy(out=c3[:], in_=C3_ps[mb][:])
                C3_sb.append(c3)
            A_sb = []
            for mb in range(2):
                asb = psm.tile([128, C], f32, tag=f"asb{mb}")
                A_sb.append(asb)
            # partition-roll of C_3 (plain direct2d copies, scalar queue —
            # safe alongside XBAR transposes)
            for mb in range(2):
                nc.scalar.dma_start(A_sb[mb][0:127, :], C3_sb[mb][1:128, :])
                nc.scalar.dma_start(A_sb[mb][127:128, :], C3_sb[1 - mb][0:1, :])

            W_acc = []
            for mb in range(2):
                wa = psm.tile([128, C], f32, tag=f"wacc{mb}")
                nc.vector.tensor_copy(out=wa[:], in_=C3_ps[mb][:])
                W_acc.append(wa)

            for mb in range(2):
                t7t = psm.tile([128, C], bf16, tag=f"t7_3_{mb}")
                nc.vector.tensor_tensor(t7t[:], A_sb[mb][:], C3_sb[mb][:], sub)
                t7[(3, mb)] = t7t

            # k loop: bmm1 C_k, then fused bmm2 partials (previous k) for ib=0
            korder = (0, 1, 2, 4, 5, 6)
            bmm2_done = []
            for idx, k in enumerate(korder):
                ck = bmm1_set(make_t3T(k))
                # after emitting C_k's matmuls, emit bmm2 partials for the
                # previous k (t7 ready) to keep tensor busy during transposes
                prev = 3 if idx == 0 else korder[idx - 1]
                bmm2_partial(prev, q_tiles0, h0s0, 0, first=(idx == 0))
                bmm2_done.append(prev)
                for mb in range(2):
                    t7t = psm.tile([128, C], bf16, tag=f"t7_{k}_{mb}")
                    nc.vector.tensor_tensor(t7t[:], A_sb[mb][:], ck[mb][:], sub)
                    t7[(k, mb)] = t7t
                    nc.vector.tensor_tensor(
                        W_acc[mb][:], W_acc[mb][:], ck[mb][:], add
                    )
            bmm2_partial(korder[-1], q_tiles0, h0s0, 0, first=False, last=True)
            bmm2_done.append(korder[-1])
            assert sorted(bmm2_done) == sorted(range(K))

            # Sm / Sj
            Sm_bf = []
            for mb in range(2):
                st = psm.tile([128, C], bf16, tag=f"smbf{mb}")
                nc.vector.scalar_tensor_tensor(
                    st[:], A_sb[mb][:], 7.0, W_acc[mb][:], mult, sub
                )
                Sm_bf.append(st)
            Sj_bf = []
            for mb in range(2):
                sj = psm.tile([128, C], bf16, tag=f"sjbf{mb}")
                Sj_bf.append(sj)
            for mb in range(2):
                nc.scalar.dma_start(Sj_bf[mb][1:128, :], Sm_bf[mb][0:127, :])
                nc.scalar.dma_start(Sj_bf[mb][0:1, :], Sm_bf[1 - mb][127:128, :])

            # ------------- finish ib=0, run ib=1, roll + store -------------
            out_sb = []
            for ib in range(2):
                osb = pf32.tile([128, H, W], f32, tag=f"osb{ib}")
                out_sb.append(osb)

            def roll_and_copy(qviews, h0s, ib, accum=False, jofs=0):
                for j, (qv, h0) in enumerate(zip(qviews, h0s)):
                    use_scalar_copy = (j + jofs) % 2 == 1 and not accum

                    def cp(o, i):
                        if accum:
                            nc.vector.tensor_tensor(o, o, i, add)
                        elif use_scalar_copy:
                            nc.scalar.copy(o, i)
                        else:
                            nc.vector.tensor_copy(out=o, in_=i)

                    def roll_copy(r0, r1, d0):
                        cp(
                            out_sb[ib][:, d0 : d0 + (r1 - r0), 0 : W - 1],
                            qv[:, r0:r1, 1:W],
                        )
                        cp(
                            out_sb[ib][:, d0 : d0 + (r1 - r0), W - 1 : W],
                            qv[:, r0:r1, 0:1],
                        )

                    if h0 + HT < H:
                        roll_copy(0, HT, h0 + 1)
                    else:
                        roll_copy(0, HT - 1, h0 + 1)
                        roll_copy(HT - 1, HT, 0)

            roll_and_copy(q_tiles0, h0s0, 0)

            # ------------- pass 2: G-only tiles (leftover ib0 + ib1) -------
            # item-sequential emission so PSUM pool rotation cannot deadlock
            items = [(0, 48)] + [(1, j * HT) for j in range(NQT)]
            for n_it, (ib, h0) in enumerate(items):
                q = pps2.tile([128, HT * W], f32, tag="ps2")
                qv = q.rearrange("p (h w) -> p h w", w=W)
                ks = (3, 0, 1, 2, 4, 5, 6)
                for idx, k in enumerate(ks):
                    bmm2_partial(
                        k, [qv], [h0], ib,
                        first=(idx == 0), last=(idx == len(ks) - 1),
                    )
                roll_and_copy([qv], [h0], ib, jofs=n_it)

            # ------------- late S-term tiles: roll-ADD into out_sb ---------
            for ib in range(2):
                for j in range(NQT):
                    h0 = j * HT
                    q = pps2.tile([128, HT * W], f32, tag="ps2")
                    qv = q.rearrange("p (h w) -> p h w", w=W)
                    bmm2_sterm([qv], [h0], ib)
                    roll_and_copy([qv], [h0], ib, accum=True)
                if ib == 0:
                    nc.scalar.dma_start(out[0:128], out_sb[0][:])
            nc.scalar.dma_start(out[128:256], out_sb[1][:])

    nc.compile()
    return nc


def _get_nc():
    if "nc" not in _CACHE:
        _CACHE["nc"] = _build_nc()
    return _CACHE["nc"]


def kernel(x: np.ndarray, p1w: np.ndarray) -> np.ndarray:
    from concourse.bass_utils import run_bass_kernel_spmd

    n = x.shape[0]
    assert n == N_CORES
    x = np.ascontiguousarray(np.asarray(x, dtype=np.float32))
    pw = np.ascontiguousarray(np.asarray(p1w, dtype=np.float32)[0])

    nc = _get_nc()
    in_maps = [{"x": x[i], "p1w": pw} for i in range(n)]
    res = run_bass_kernel_spmd(nc, in_maps, list(range(N_CORES)))
    outs = [res.results[i]["out"] for i in range(n)]
    return np.stack(outs, axis=0).astype(np.float32)


# revision 18
# speedup vs baseline: 1.0709x; 1.0709x over previous
"""Trainium2 Bass kernel for the sparse_attention nn_Kernel problem.

Math (per sample, derived from the reference):
  t1 = p1w * x ; t2 = roll(t1, 1, ch) ; t3_k = shift_{k-3}(t2) (zero-padded, w)
  C_k[i,m] = sum_p x[i,p] * t1pad[m, p+s]   (s = k-3; c x c)
  t7m_k[q=m, i] = A[q] - C_k^T[q]  where A[q] = C_3^T[q+1] (partition roll)
  Sm[q=m] = 7*A[q] - sum_k C_k^T[q] ; Sj[q] = Sm[q-1]
  out = roll_{h+1,w-1}( sum_k t7m_k^T @ t1pad(col shift s) + Sj^T @ invroll_hw(x) )

Key layout/schedule tricks vs the original baseline:
  - the 7 shifted transposed operands t3_kT[p',m] = t1pad[m, p'+s] are built by
    XBAR DMA transposes reading the t1 buffer at a free-dim column offset s
    (slack zeros give exact unfold semantics), NOT via a DRAM partition-shift
    bounce.  This halves DMA traffic and removes ~120us of serialized queue
    time.  All XBAR transposes stay on ONE queue (sync): concurrent XBAR
    transposes on both hwdge queues corrupt each other (shared XBAR unit).
  - plain DRAM loads/stores and small partition-roll copies go on the OTHER
    hwdge queue (scalar) and overlap the XBAR phase safely.
  - bmm2's ib=0 accumulation is interleaved into the bmm1 k-loop (7 PSUM
    Q-tiles stay open across k), so the XBAR-transpose-bound phase also
    retires bmm2 work; ib=1 runs as a second pass from the persistent t7
    tiles.
  - pad-only memsets, issued before the loads; element-wise work split
    across vector / scalar / gpsimd; final cyclic roll folded into the
    PSUM->SBUF copies.

Each of the 8 cores processes one sample of the batch (data parallel).
"""

import math

import numpy as np

C = 256
H = 56
W = 56
WP = 64  # padded width
PADL = 3
NPP = H * WP  # 3584 padded positions
NCH = NPP // 128  # 28 chunks of 128 partitions
K = 7
SL = 8  # slack zero cols each side of t1buf so shifted transposes stay in range
BETA = 1.0 / (math.sqrt(H * W) * math.sqrt(C * K))
N_CORES = 8
HT = 8  # h rows per bmm2 out tile
NQT = H // HT  # 7 tiles per ib

_CACHE = {}


def _build_nc():
    import concourse.mybir as mybir
    import concourse.tile as tile
    from concourse import bacc

    f32 = mybir.dt.float32
    bf16 = mybir.dt.bfloat16

    nc = bacc.Bacc("TRN2", target_bir_lowering=False, debug=False)

    xin = nc.dram_tensor("x", [C, H, W], f32, kind="ExternalInput").ap()
    pwin = nc.dram_tensor("p1w", [C, H, W], f32, kind="ExternalInput").ap()
    out = nc.dram_tensor("out", [C, H, W], f32, kind="ExternalOutput").ap()

    sub = mybir.AluOpType.subtract
    mult = mybir.AluOpType.mult
    add = mybir.AluOpType.add

    with tile.TileContext(nc) as tc:
        with (
            tc.tile_pool(name="f32big", bufs=1) as pf32,
            tc.tile_pool(name="bfbig", bufs=1) as pbf,
            tc.tile_pool(name="bfroll", bufs=1) as pbr,
            tc.tile_pool(name="ptrans", bufs=1) as pxT,
            tc.tile_pool(name="pt3", bufs=3) as pt3,
            tc.tile_pool(name="small", bufs=1) as psm,
            tc.tile_pool(name="ps1", bufs=2, space="PSUM") as pps1,
            tc.tile_pool(name="ps2", bufs=6, space="PSUM") as pps2,
        ):
            # ------------- tiles + pad memsets (before loads) -------------
            x_cp, p_cp, x_bf, t1buf = [], [], [], []
            for cb in range(2):
                xt = pf32.tile([128, H, W], f32, tag=f"xcp{cb}")
                x_cp.append(xt)
                pt = pf32.tile([128, H, W], f32, tag=f"pcp{cb}")
                p_cp.append(pt)

                tb = pbf.tile([128, 2 * SL + NPP], bf16, tag=f"t1b{cb}")
                nc.vector.memset(tb[:, 0:SL], 0.0)
                nc.vector.memset(tb[:, SL + NPP : 2 * SL + NPP], 0.0)
                tb3 = tb[:, SL : SL + NPP].rearrange("p (h w) -> p h w", w=WP)
                nc.vector.memset(tb3[:, :, 0:PADL], 0.0)
                nc.vector.memset(tb3[:, :, PADL + W : WP], 0.0)
                t1buf.append(tb)

                xb = pbf.tile([128, NPP], bf16, tag=f"xbf{cb}")
                xb3 = xb.rearrange("p (h w) -> p h w", w=WP)
                nc.vector.memset(xb3[:, :, 0:PADL], 0.0)
                nc.vector.memset(xb3[:, :, PADL + W : WP], 0.0)
                x_bf.append(xb)

            # ------------- loads (scalar hwdge queue; sync = XBAR only) ----
            for cb in range(2):
                nc.sync.dma_start(x_cp[cb][:], xin[cb * 128 : (cb + 1) * 128])
                nc.scalar.dma_start(p_cp[cb][:], pwin[cb * 128 : (cb + 1) * 128])

            # ------------- padded bf16 operands ----------------------------
            for cb in range(2):
                tb3 = t1buf[cb][:, SL : SL + NPP].rearrange(
                    "p (h w) -> p h w", w=WP
                )
                nc.vector.tensor_mul(
                    tb3[:, :, PADL : PADL + W], x_cp[cb][:], p_cp[cb][:]
                )
                xb3 = x_bf[cb].rearrange("p (h w) -> p h w", w=WP)
                nc.vector.tensor_scalar_mul(
                    xb3[:, :, PADL : PADL + W], x_cp[cb][:], BETA
                )

            def t1view(cb):
                return t1buf[cb][:, SL : SL + NPP].rearrange(
                    "p (h w) -> p h w", w=WP
                )

            # ------------- transposes (XBAR, sync queue ONLY) --------------
            t1T = pxT.tile([128, NCH, C], bf16, tag="t1T")
            xpT = pxT.tile([128, NCH, C], bf16, tag="xpT")
            for cb in range(2):
                nc.sync.dma_start_transpose(
                    t1T[:, :, cb * 128 : (cb + 1) * 128],
                    t1buf[cb][:, SL : SL + NPP],
                )
                nc.sync.dma_start_transpose(
                    xpT[:, :, cb * 128 : (cb + 1) * 128], x_bf[cb][:, :]
                )

            def make_t3T(k):
                s = k - 3
                t3k = pt3.tile([128, NCH, C], bf16, tag="t3")
                for cb in range(2):
                    nc.sync.dma_start_transpose(
                        t3k[:, :, cb * 128 : (cb + 1) * 128],
                        t1buf[cb][:, SL + s : SL + s + NPP],
                    )
                return t3k

            # xroll[j, h', w'] = x[j, (h'+1)%H, (w'-1)%W]  (bf16, for S-term;
            # vector, after the transpose-critical prep)
            xroll = []
            for cb in range(2):
                xr = pbr.tile([128, H, W], bf16, tag=f"xroll{cb}")
                nc.vector.tensor_copy(
                    out=xr[:, 0:55, 1:W], in_=x_cp[cb][:, 1:56, 0 : W - 1]
                )
                nc.vector.tensor_copy(
                    out=xr[:, 0:55, 0:1], in_=x_cp[cb][:, 1:56, W - 1 : W]
                )
                nc.vector.tensor_copy(
                    out=xr[:, 55:56, 1:W], in_=x_cp[cb][:, 0:1, 0 : W - 1]
                )
                nc.vector.tensor_copy(
                    out=xr[:, 55:56, 0:1], in_=x_cp[cb][:, 0:1, W - 1 : W]
                )
                xroll.append(xr)

            # ------------- bmm1 set helper ---------------------------------
            def bmm1_set(Tw):
                tiles = []
                for mb in range(2):
                    pt = pps1.tile([128, C], f32, tag="ps1")
                    for t in range(NCH):
                        nc.tensor.matmul(
                            pt[:],
                            Tw[:, t, mb * 128 : mb * 128 + 128],
                            xpT[:, t, :],
                            start=(t == 0),
                            stop=(t == NCH - 1),
                        )
                    tiles.append(pt)
                return tiles

            # ------------- bmm2 ib=0 Q tiles (persist across the k loop) ---
            NQ0 = 6  # tiles fused into the k loop (PSUM bank budget)
            q_tiles0 = []
            for j in range(NQ0):
                q = pps2.tile([128, HT * W], f32, tag="ps2")
                q_tiles0.append(q.rearrange("p (h w) -> p h w", w=W))

            h0s0 = [j * HT for j in range(NQ0)]
            t7 = {}

            def bmm2_partial(k, qviews, h0s, ib, first, last=False):
                s = k - 3
                for mb in range(2):
                    for qi, (qv, h0) in enumerate(zip(qviews, h0s)):
                        rhs = t1view(mb)[
                            :, h0 : h0 + HT, PADL + s : PADL + s + W
                        ]
                        nc.tensor.matmul(
                            qv[:],
                            t7[(k, mb)][:, ib * 128 : (ib + 1) * 128],
                            rhs,
                            start=(first and mb == 0),
                            stop=(last and mb == 1),
                        )

            def bmm2_sterm(qviews, h0s, ib):
                for mb in range(2):
                    for qv, h0 in zip(qviews, h0s):
                        nc.tensor.matmul(
                            qv[:],
                            Sj_bf[mb][:, ib * 128 : (ib + 1) * 128],
                            xroll[mb][:, h0 : h0 + HT, :],
                            start=(mb == 0),
                            stop=(mb == 1),
                        )

            # ------------- C_3, A roll, then k loop with fused bmm2 --------
            C3_ps = bmm1_set(t1T)
            C3_sb = []
            for mb in range(2):
                c3 = psm.tile([128, C], f32, tag=f"c3sb{mb}")
                nc.vector.tensor_copy(out=c3[:], in_=C3_ps[mb][:])
                C3_sb.append(c3)
            A_sb = []
            for mb in range(2):
                asb = psm.tile([128, C], f32, tag=f"asb{mb}")
                A_sb.append(asb)
            # partition-roll of C_3 (plain direct2d copies, scalar queue —
            # safe alongside XBAR transposes)
            for mb in range(2):
                nc.scalar.dma_start(A_sb[mb][0:127, :], C3_sb[mb][1:128, :])
                nc.scalar.dma_start(A_sb[mb][127:128, :], C3_sb[1 - mb][0:1, :])

            W_acc = []
            for mb in range(2):
                wa = psm.tile([128, C], f32, tag=f"wacc{mb}")
                nc.vector.tensor_copy(out=wa[:], in_=C3_ps[mb][:])
                W_acc.append(wa)

            for mb in range(2):
                t7t = psm.tile([128, C], bf16, tag=f"t7_3_{mb}")
                nc.vector.tensor_tensor(t7t[:], A_sb[mb][:], C3_sb[mb][:], sub)
                t7[(3, mb)] = t7t

            # k loop: bmm1 C_k, then fused bmm2 partials (previous k) for ib=0
            korder = (0, 1, 2, 4, 5, 6)
            bmm2_done = []
            for idx, k in enumerate(korder):
                ck = bmm1_set(make_t3T(k))
                # after emitting C_k's matmuls, emit bmm2 partials for the
                # previous k (t7 ready) to keep tensor busy during transposes
                prev = 3 if idx == 0 else korder[idx - 1]
                bmm2_partial(prev, q_tiles0, h0s0, 0, first=(idx == 0))
                bmm2_done.append(prev)
                for mb in range(2):
                    t7t = psm.tile([128, C], bf16, tag=f"t7_{k}_{mb}")
                    nc.vector.tensor_tensor(t7t[:], A_sb[mb][:], ck[mb][:], sub)
                    t7[(k, mb)] = t7t
                    nc.vector.tensor_tensor(
                        W_acc[mb][:], W_acc[mb][:], ck[mb][:], add
                    )
            bmm2_partial(korder[-1], q_tiles0, h0s0, 0, first=False, last=True)
            bmm2_done.append(korder[-1])
            assert sorted(bmm2_done) == sorted(range(K))

            # Sm / Sj
            Sm_bf = []
            for mb in range(2):
                st = psm.tile([128, C], bf16, tag=f"smbf{mb}")
                nc.vector.scalar_tensor_tensor(
                    st[:], A_sb[mb][:], 7.0, W_acc[mb][:], mult, sub
                )
                Sm_bf.append(st)
            Sj_bf = []
            for mb in range(2):
                sj = psm.tile([128, C], bf16, tag=f"sjbf{mb}")
                Sj_bf.append(sj)
            for mb in range(2):
                nc.scalar.dma_start(Sj_bf[mb][1:128, :], Sm_bf[mb][0:127, :])
                nc.scalar.dma_start(Sj_bf[mb][0:1, :], Sm_bf[1 - mb][127:128, :])

            # ------------- finish ib=0, run ib=1, roll + store -------------
            out_sb = []
            for ib in range(2):
                osb = pf32.tile([128, H, W], f32, tag=f"osb{ib}")
                out_sb.append(osb)

            def roll_and_copy(qviews, h0s, ib, accum=False, jofs=0):
                for j, (qv, h0) in enumerate(zip(qviews, h0s)):
                    use_scalar_copy = (j + jofs) % 2 == 1 and not accum

                    def cp(o, i):
                        if accum:
                            nc.vector.tensor_tensor(o, o, i, add)
                        elif use_scalar_copy:
                            nc.scalar.copy(o, i)
                        else:
                            nc.vector.tensor_copy(out=o, in_=i)

                    def roll_copy(r0, r1, d0):
                        cp(
                            out_sb[ib][:, d0 : d0 + (r1 - r0), 0 : W - 1],
                            qv[:, r0:r1, 1:W],
                        )
                        cp(
                            out_sb[ib][:, d0 : d0 + (r1 - r0), W - 1 : W],
                            qv[:, r0:r1, 0:1],
                        )

                    if h0 + HT < H:
                        roll_copy(0, HT, h0 + 1)
                    else:
                        roll_copy(0, HT - 1, h0 + 1)
                        roll_copy(HT - 1, HT, 0)

            roll_and_copy(q_tiles0, h0s0, 0)

            # ------------- pass 2: G-only tiles (leftover ib0 + ib1) -------
            # item-sequential emission so PSUM pool rotation cannot deadlock
            items = [(0, 48)] + [(1, j * HT) for j in range(NQT)]
            for n_it, (ib, h0) in enumerate(items):
                q = pps2.tile([128, HT * W], f32, tag="ps2")
                qv = q.rearrange("p (h w) -> p h w", w=W)
                ks = (3, 0, 1, 2, 4, 5, 6)
                for idx, k in enumerate(ks):
                    bmm2_partial(
                        k, [qv], [h0], ib,
                        first=(idx == 0), last=(idx == len(ks) - 1),
                    )
                roll_and_copy([qv], [h0], ib, jofs=n_it)

            # ------------- late S-term tiles: roll-ADD into out_sb, then
            # stream the finalized 8-row pieces to DRAM on both queues ------
            np_q = 0
            for ib in range(2):
                ob = out[ib * 128 : (ib + 1) * 128]
                for j in range(NQT):
                    h0 = j * HT
                    q = pps2.tile([128, HT * W], f32, tag="ps2")
                    qv = q.rearrange("p (h w) -> p h w", w=W)
                    bmm2_sterm([qv], [h0], ib)
                    roll_and_copy([qv], [h0], ib, accum=True)
                    eng = nc.sync if np_q % 2 == 0 else nc.scalar
                    np_q += 1
                    if h0 + HT < H:
                        eng.dma_start(
                            ob[:, h0 + 1 : h0 + 1 + HT, :],
                            out_sb[ib][:, h0 + 1 : h0 + 1 + HT, :],
                        )
                    else:
                        eng.dma_start(
                            ob[:, h0 + 1 : H, :],
                            out_sb[ib][:, h0 + 1 : H, :],
                        )
                        eng.dma_start(
                            ob[:, 0:1, :], out_sb[ib][:, 0:1, :]
                        )

    nc.compile()
    return nc


def _get_nc():
    if "nc" not in _CACHE:
        _CACHE["nc"] = _build_nc()
    return _CACHE["nc"]


def kernel(x: np.ndarray, p1w: np.ndarray) -> np.ndarray:
    from concourse.bass_utils import run_bass_kernel_spmd

    n = x.shape[0]
    assert n == N_CORES
    x = np.ascontiguousarray(np.asarray(x, dtype=np.float32))
    pw = np.ascontiguousarray(np.asarray(p1w, dtype=np.float32)[0])

    nc = _get_nc()
    in_maps = [{"x": x[i], "p1w": pw} for i in range(n)]
    res = run_bass_kernel_spmd(nc, in_maps, list(range(N_CORES)))
    outs = [res.results[i]["out"] for i in range(n)]
    return np.stack(outs, axis=0).astype(np.float32)


# revision 20
# speedup vs baseline: 1.1125x; 1.0389x over previous
"""Trainium2 Bass kernel for the sparse_attention nn_Kernel problem.

Math (per sample, derived from the reference):
  t1 = p1w * x ; t2 = roll(t1, 1, ch) ; t3_k = shift_{k-3}(t2) (zero-padded, w)
  C_k[i,m] = sum_p x[i,p] * t1pad[m, p+s]   (s = k-3; c x c)
  t7m_k[q=m, i] = A[q] - C_k^T[q]  where A[q] = C_3^T[q+1] (partition roll)
  Sm[q=m] = 7*A[q] - sum_k C_k^T[q] ; Sj[q] = Sm[q-1]
  out = roll_{h+1,w-1}( sum_k t7m_k^T @ t1pad(col shift s) + Sj^T @ invroll_hw(x) )

Key layout/schedule tricks vs the original baseline:
  - the 7 shifted transposed operands t3_kT[p',m] = t1pad[m, p'+s] are built by
    XBAR DMA transposes reading the t1 buffer at a free-dim column offset s
    (slack zeros give exact unfold semantics), NOT via a DRAM partition-shift
    bounce.  This halves DMA traffic and removes ~120us of serialized queue
    time.  All XBAR transposes stay on ONE queue (sync): concurrent XBAR
    transposes on both hwdge queues corrupt each other (shared XBAR unit).
  - plain DRAM loads/stores and small partition-roll copies go on the OTHER
    hwdge queue (scalar) and overlap the XBAR phase safely.
  - bmm2's ib=0 accumulation is interleaved into the bmm1 k-loop (7 PSUM
    Q-tiles stay open across k), so the XBAR-transpose-bound phase also
    retires bmm2 work; ib=1 runs as a second pass from the persistent t7
    tiles.
  - pad-only memsets, issued before the loads; element-wise work split
    across vector / scalar / gpsimd; final cyclic roll folded into the
    PSUM->SBUF copies.

Each of the 8 cores processes one sample of the batch (data parallel).
"""

import math

import numpy as np

C = 256
H = 56
W = 56
WP = 64  # padded width
PADL = 3
NPP = H * WP  # 3584 padded positions
NCH = NPP // 128  # 28 chunks of 128 partitions
K = 7
SL = 8  # slack zero cols each side of t1buf so shifted transposes stay in range
BETA = 1.0 / (math.sqrt(H * W) * math.sqrt(C * K))
N_CORES = 8
HT = 8  # h rows per bmm2 out tile
NQT = H // HT  # 7 tiles per ib

_CACHE = {}


def _build_nc():
    import concourse.mybir as mybir
    import concourse.tile as tile
    from concourse import bacc

    f32 = mybir.dt.float32
    bf16 = mybir.dt.bfloat16

    nc = bacc.Bacc("TRN2", target_bir_lowering=False, debug=False)

    xin = nc.dram_tensor("x", [C, H, W], f32, kind="ExternalInput").ap()
    pwin = nc.dram_tensor("p1w", [C, H, W], f32, kind="ExternalInput").ap()
    out = nc.dram_tensor("out", [C, H, W], f32, kind="ExternalOutput").ap()

    sub = mybir.AluOpType.subtract
    mult = mybir.AluOpType.mult
    add = mybir.AluOpType.add

    with tile.TileContext(nc) as tc:
        with (
            tc.tile_pool(name="f32big", bufs=1) as pf32,
            tc.tile_pool(name="bfbig", bufs=1) as pbf,
            tc.tile_pool(name="bfroll", bufs=1) as pbr,
            tc.tile_pool(name="ptrans", bufs=1) as pxT,
            tc.tile_pool(name="pt3", bufs=3) as pt3,
            tc.tile_pool(name="small", bufs=1) as psm,
            tc.tile_pool(name="ps1", bufs=2, space="PSUM") as pps1,
            tc.tile_pool(name="ps2", bufs=6, space="PSUM") as pps2,
        ):
            # ------------- tiles + pad memsets (before loads) -------------
            x_cp, p_cp, x_bf, t1buf = [], [], [], []
            for cb in range(2):
                xt = pf32.tile([128, H, W], f32, tag=f"xcp{cb}")
                x_cp.append(xt)
                pt = pf32.tile([128, H, W], f32, tag=f"pcp{cb}")
                p_cp.append(pt)

                tb = pbf.tile([128, 2 * SL + NPP], bf16, tag=f"t1b{cb}")
                nc.vector.memset(tb[:, 0:SL], 0.0)
                nc.vector.memset(tb[:, SL + NPP : 2 * SL + NPP], 0.0)
                tb3 = tb[:, SL : SL + NPP].rearrange("p (h w) -> p h w", w=WP)
                nc.vector.memset(tb3[:, :, 0:PADL], 0.0)
                nc.vector.memset(tb3[:, :, PADL + W : WP], 0.0)
                t1buf.append(tb)

                xb = pbf.tile([128, NPP], bf16, tag=f"xbf{cb}")
                xb3 = xb.rearrange("p (h w) -> p h w", w=WP)
                nc.vector.memset(xb3[:, :, 0:PADL], 0.0)
                nc.vector.memset(xb3[:, :, PADL + W : WP], 0.0)
                x_bf.append(xb)

            # ------------- loads (scalar hwdge queue; sync = XBAR only) ----
            for cb in range(2):
                nc.sync.dma_start(x_cp[cb][:], xin[cb * 128 : (cb + 1) * 128])
                nc.scalar.dma_start(p_cp[cb][:], pwin[cb * 128 : (cb + 1) * 128])

            # ------------- padded bf16 operands ----------------------------
            for cb in range(2):
                tb3 = t1buf[cb][:, SL : SL + NPP].rearrange(
                    "p (h w) -> p h w", w=WP
                )
                nc.vector.tensor_mul(
                    tb3[:, :, PADL : PADL + W], x_cp[cb][:], p_cp[cb][:]
                )
                xb3 = x_bf[cb].rearrange("p (h w) -> p h w", w=WP)
                nc.vector.tensor_scalar_mul(
                    xb3[:, :, PADL : PADL + W], x_cp[cb][:], BETA
                )

            def t1view(cb):
                return t1buf[cb][:, SL : SL + NPP].rearrange(
                    "p (h w) -> p h w", w=WP
                )

            # ------------- transposes (XBAR, sync queue ONLY) --------------
            t1T = pxT.tile([128, NCH, C], bf16, tag="t1T")
            xpT = pxT.tile([128, NCH, C], bf16, tag="xpT")
            for cb in range(2):
                nc.sync.dma_start_transpose(
                    t1T[:, :, cb * 128 : (cb + 1) * 128],
                    t1buf[cb][:, SL : SL + NPP],
                )
                nc.sync.dma_start_transpose(
                    xpT[:, :, cb * 128 : (cb + 1) * 128], x_bf[cb][:, :]
                )

            def make_t3T(k):
                s = k - 3
                t3k = pt3.tile([128, NCH, C], bf16, tag="t3")
                for cb in range(2):
                    nc.sync.dma_start_transpose(
                        t3k[:, :, cb * 128 : (cb + 1) * 128],
                        t1buf[cb][:, SL + s : SL + s + NPP],
                    )
                return t3k

            # xroll[j, h', w'] = x[j, (h'+1)%H, (w'-1)%W]  (bf16, for S-term;
            # vector, after the transpose-critical prep)
            xroll = []
            for cb in range(2):
                xr = pbr.tile([128, H, W], bf16, tag=f"xroll{cb}")
                nc.vector.tensor_copy(
                    out=xr[:, 0:55, 1:W], in_=x_cp[cb][:, 1:56, 0 : W - 1]
                )
                nc.vector.tensor_copy(
                    out=xr[:, 0:55, 0:1], in_=x_cp[cb][:, 1:56, W - 1 : W]
                )
                nc.vector.tensor_copy(
                    out=xr[:, 55:56, 1:W], in_=x_cp[cb][:, 0:1, 0 : W - 1]
                )
                nc.vector.tensor_copy(
                    out=xr[:, 55:56, 0:1], in_=x_cp[cb][:, 0:1, W - 1 : W]
                )
                xroll.append(xr)

            # ------------- bmm1 set helper ---------------------------------
            def bmm1_set(Tw):
                tiles = []
                for mb in range(2):
                    pt = pps1.tile([128, C], f32, tag="ps1")
                    for t in range(NCH):
                        nc.tensor.matmul(
                            pt[:],
                            Tw[:, t, mb * 128 : mb * 128 + 128],
                            xpT[:, t, :],
                            start=(t == 0),
                            stop=(t == NCH - 1),
                        )
                    tiles.append(pt)
                return tiles

            # ------------- bmm2 ib=0 Q tiles (persist across the k loop) ---
            NQ0 = 6  # tiles fused into the k loop (PSUM bank budget)
            q_tiles0 = []
            for j in range(NQ0):
                q = pps2.tile([128, HT * W], f32, tag="ps2")
                q_tiles0.append(q.rearrange("p (h w) -> p h w", w=W))

            h0s0 = [j * HT for j in range(NQ0)]
            t7 = {}

            def bmm2_partial(k, qviews, h0s, ib, first, last=False):
                s = k - 3
                for mb in range(2):
                    for qi, (qv, h0) in enumerate(zip(qviews, h0s)):
                        rhs = t1view(mb)[
                            :, h0 : h0 + HT, PADL + s : PADL + s + W
                        ]
                        nc.tensor.matmul(
                            qv[:],
                            t7[(k, mb)][:, ib * 128 : (ib + 1) * 128],
                            rhs,
                            start=(first and mb == 0),
                            stop=(last and mb == 1),
                        )

            def bmm2_sterm(qviews, h0s, ib):
                for mb in range(2):
                    for qv, h0 in zip(qviews, h0s):
                        nc.tensor.matmul(
                            qv[:],
                            Sj_bf[mb][:, ib * 128 : (ib + 1) * 128],
                            xroll[mb][:, h0 : h0 + HT, :],
                            start=(mb == 0),
                            stop=(mb == 1),
                        )

            # ------------- C_3, A roll, then k loop with fused bmm2 --------
            C3_ps = bmm1_set(t1T)
            C3_sb = []
            for mb in range(2):
                c3 = psm.tile([128, C], f32, tag=f"c3sb{mb}")
                nc.vector.tensor_copy(out=c3[:], in_=C3_ps[mb][:])
                C3_sb.append(c3)
            A_sb = []
            for mb in range(2):
                asb = psm.tile([128, C], f32, tag=f"asb{mb}")
                A_sb.append(asb)

            def emit_a_rolls():
                # partition-roll of C_3 (small direct2d copies on sync,
                # emitted late so they never gate the transpose stream)
                for mb in range(2):
                    nc.sync.dma_start(A_sb[mb][0:127, :], C3_sb[mb][1:128, :])
                    nc.sync.dma_start(
                        A_sb[mb][127:128, :], C3_sb[1 - mb][0:1, :]
                    )

            W_acc = []
            for mb in range(2):
                wa = psm.tile([128, C], f32, tag=f"wacc{mb}")
                nc.vector.tensor_copy(out=wa[:], in_=C3_ps[mb][:])
                W_acc.append(wa)

            # k loop: bmm1 C_k, then fused bmm2 partials (previous k) for ib=0
            korder = (0, 1, 2, 4, 5, 6)
            bmm2_done = []
            for idx, k in enumerate(korder):
                t3T = make_t3T(k)
                if idx == 0:
                    emit_a_rolls()
                    # t7 for k=3 (needs A): emitted after the A-roll writes
                    for mb in range(2):
                        t7t = psm.tile([128, C], bf16, tag=f"t7_3_{mb}")
                        nc.vector.tensor_tensor(
                            t7t[:], A_sb[mb][:], C3_sb[mb][:], sub
                        )
                        t7[(3, mb)] = t7t
                ck = bmm1_set(t3T)
                # after emitting C_k's matmuls, emit bmm2 partials for the
                # previous k (t7 ready) to keep tensor busy during transposes
                prev = 3 if idx == 0 else korder[idx - 1]
                bmm2_partial(prev, q_tiles0, h0s0, 0, first=(idx == 0))
                bmm2_done.append(prev)
                for mb in range(2):
                    t7t = psm.tile([128, C], bf16, tag=f"t7_{k}_{mb}")
                    nc.vector.tensor_tensor(t7t[:], A_sb[mb][:], ck[mb][:], sub)
                    t7[(k, mb)] = t7t
                    nc.vector.tensor_tensor(
                        W_acc[mb][:], W_acc[mb][:], ck[mb][:], add
                    )
            bmm2_partial(korder[-1], q_tiles0, h0s0, 0, first=False, last=True)
            bmm2_done.append(korder[-1])
            assert sorted(bmm2_done) == sorted(range(K))

            # Sm / Sj
            Sm_bf = []
            for mb in range(2):
                st = psm.tile([128, C], bf16, tag=f"smbf{mb}")
                nc.vector.scalar_tensor_tensor(
                    st[:], A_sb[mb][:], 7.0, W_acc[mb][:], mult, sub
                )
                Sm_bf.append(st)
            Sj_bf = []
            for mb in range(2):
                sj = psm.tile([128, C], bf16, tag=f"sjbf{mb}")
                Sj_bf.append(sj)
            for mb in range(2):
                nc.sync.dma_start(Sj_bf[mb][1:128, :], Sm_bf[mb][0:127, :])
                nc.sync.dma_start(Sj_bf[mb][0:1, :], Sm_bf[1 - mb][127:128, :])

            # ------------- finish ib=0, run ib=1, roll + store -------------
            out_sb = []
            for ib in range(2):
                osb = pf32.tile([128, H, W], f32, tag=f"osb{ib}")
                out_sb.append(osb)

            def roll_and_copy(qviews, h0s, ib, accum=False, jofs=0):
                for j, (qv, h0) in enumerate(zip(qviews, h0s)):
                    use_scalar_copy = (j + jofs) % 2 == 1 and not accum

                    def cp(o, i):
                        if accum:
                            nc.vector.tensor_tensor(o, o, i, add)
                        elif use_scalar_copy:
                            nc.scalar.copy(o, i)
                        else:
                            nc.vector.tensor_copy(out=o, in_=i)

                    def roll_copy(r0, r1, d0):
                        cp(
                            out_sb[ib][:, d0 : d0 + (r1 - r0), 0 : W - 1],
                            qv[:, r0:r1, 1:W],
                        )
                        cp(
                            out_sb[ib][:, d0 : d0 + (r1 - r0), W - 1 : W],
                            qv[:, r0:r1, 0:1],
                        )

                    if h0 + HT < H:
                        roll_copy(0, HT, h0 + 1)
                    else:
                        roll_copy(0, HT - 1, h0 + 1)
                        roll_copy(HT - 1, HT, 0)

            roll_and_copy(q_tiles0, h0s0, 0)

            # ------------- pass 2: G-only tiles (leftover ib0 + ib1) -------
            # item-sequential emission so PSUM pool rotation cannot deadlock
            items = [(0, 48)] + [(1, j * HT) for j in range(NQT)]
            for n_it, (ib, h0) in enumerate(items):
                q = pps2.tile([128, HT * W], f32, tag="ps2")
                qv = q.rearrange("p (h w) -> p h w", w=W)
                ks = (3, 0, 1, 2, 4, 5, 6)
                for idx, k in enumerate(ks):
                    bmm2_partial(
                        k, [qv], [h0], ib,
                        first=(idx == 0), last=(idx == len(ks) - 1),
                    )
                roll_and_copy([qv], [h0], ib, jofs=n_it)

            # ------------- late S-term tiles: roll-ADD into out_sb, then
            # stream the finalized 8-row pieces to DRAM on both queues ------
            np_q = 0
            for ib in range(2):
                ob = out[ib * 128 : (ib + 1) * 128]
                for j in range(NQT):
                    h0 = j * HT
                    q = pps2.tile([128, HT * W], f32, tag="ps2")
                    qv = q.rearrange("p (h w) -> p h w", w=W)
                    bmm2_sterm([qv], [h0], ib)
                    roll_and_copy([qv], [h0], ib, accum=True)
                    eng = nc.sync if np_q % 2 == 0 else nc.scalar
                    np_q += 1
                    if h0 + HT < H:
                        eng.dma_start(
                            ob[:, h0 + 1 : h0 + 1 + HT, :],
                            out_sb[ib][:, h0 + 1 : h0 + 1 + HT, :],
                        )
                    else:
                        eng.dma_start(
                            ob[:, h0 + 1 : H, :],
                            out_sb[ib][:, h0 + 1 : H, :],
                        )
                        eng.dma_start(
                            ob[:, 0:1, :], out_sb[ib][:, 0:1, :]
                        )

    nc.compile()
    return nc


def _get_nc():
    if "nc" not in _CACHE:
        _CACHE["nc"] = _build_nc()
    return _CACHE["nc"]


def kernel(x: np.ndarray, p1w: np.ndarray) -> np.ndarray:
    from concourse.bass_utils import run_bass_kernel_spmd

    n = x.shape[0]
    assert n == N_CORES
    x = np.ascontiguousarray(np.asarray(x, dtype=np.float32))
    pw = np.ascontiguousarray(np.asarray(p1w, dtype=np.float32)[0])

    nc = _get_nc()
    in_maps = [{"x": x[i], "p1w": pw} for i in range(n)]
    res = run_bass_kernel_spmd(nc, in_maps, list(range(N_CORES)))
    outs = [res.results[i]["out"] for i in range(n)]
    return np.stack(outs, axis=0).astype(np.float32)
